# revision 4
# baseline (speedup 1.0000x reference)
"""Trainium2 Bass kernel for nn_CentroidLoss (BCE + sparse-centroid selem similarity).

Full inputs in, full (scalar) output out. Sharding: flattened voxel axis N
split contiguously across 8 cores (one D-slice each).

Math: loss = mean_c BCE(x_c, t_c) + 0.5*mean(sims[:3]) + 0.5*(1-sims[3]) where
sims_c = (1/n_cent) * sum_i cm_i * (sum_k w_k valid x_c[i+off_k]) / cnt_i.
The centroid mask cm is ~0.01% dense, so the double sum is re-associated as
dot(x_c, A) with A[j] = sum_{i,k: i+off_k=j} cm_i * w_k / cnt_i, a sparse
scatter computed on host from the mask (~82*243 scalar ops). The device
streams x, t and A once (memory-roofline) and emits per-partition partial
sums; host combines the 8*128 partials into the scalar loss.
"""

import os
import numpy as np

import concourse.bass as bass
import concourse.mybir as mybir
from concourse.tile import TileContext
from concourse import bass_utils

# ---- hardcoded problem geometry ----
D, H, W = 8, 320, 320
N = D * H * W                      # 819200
NCORES = 8
CHUNK = N // NCORES                # 102400
P = 128
F = CHUNK // P                     # 800
CH = 4
EPS = 1e-7
ETA = 0.5
PHI = 0.5
CHAN_WEIGHTS = (1.0, 1.0, 1.0)

SELEM_SHAPE = (3, 9, 9)
CENTRE = (1, 4, 4)

_cache = {}


def _split_multi_waits(nc):
    """This walrus build rejects >1 sync-wait per instruction ("Too many sync
    wait commands"). Tile coalesces waits; redistribute extras onto NoOps
    inserted immediately before, on the same engine (engine blocks on each
    wait in turn — semantics preserved)."""
    n_split = 0
    for fn in nc.m.functions:
        for b in fn.blocks:
            insts = b.instructions
            i = 0
            while i < len(insts):
                inst = insts[i]
                si = getattr(inst, 'sync_info', None)
                if si is None or not si.on_wait or len(si.on_wait) <= 1:
                    i += 1
                    continue
                waits = list(si.on_wait)
                new_nops = [
                    mybir.InstNoOp(
                        name=f"{inst.name}-waitsplit-{k}",
                        engine=inst.engine,
                        sync_info=mybir.SyncInfo(on_wait=[w], on_update=[]),
                    )
                    for k, w in enumerate(waits[:-1])
                ]
                si.on_wait = [waits[-1]]
                for k, nop in enumerate(new_nops):
                    insts.insert(i + k, nop)
                i += len(new_nops) + 1
                n_split += 1
    return n_split


def _offsets_and_weights():
    idx = np.stack(np.nonzero(np.ones(SELEM_SHAPE)), axis=-1)      # (243, 3)
    disp = idx - np.asarray(CENTRE)
    strides = np.array([H * W, W, 1])
    offsets = disp @ strides                                        # (243,)
    dist = np.linalg.norm(disp.astype(np.float64), axis=1)
    weights = (dist / dist.max() - 1.0).astype(np.float32)          # (243,)
    return offsets.astype(np.int64), weights


def _build_nc():
    nc = bass.Bass()
    f32 = mybir.dt.float32
    x = nc.dram_tensor("x", (CH, P, F), f32, kind="ExternalInput")
    t = nc.dram_tensor("t", (CH, P, F), f32, kind="ExternalInput")
    a = nc.dram_tensor("a", (P, F), f32, kind="ExternalInput")
    out = nc.dram_tensor("out", (P, 12), f32, kind="ExternalOutput")
    Ln = mybir.ActivationFunctionType.Ln
    Al = mybir.AluOpType

    with TileContext(nc) as tc:
        with tc.tile_pool(name="pool", bufs=2) as pool:
            o = pool.tile([P, 12], f32, bufs=1)
            a_t = pool.tile([P, F], f32, bufs=1)
            nc.sync.dma_start(out=a_t[:], in_=a[:, :])
            junk = pool.tile([P, F], f32, bufs=1)
            for c in range(CH):
                x_t = pool.tile([P, F], f32, tag="x", bufs=3)
                nc.sync.dma_start(out=x_t[:], in_=x[c, :, :])
                t_t = pool.tile([P, F], f32, tag="t", bufs=3)
                nc.sync.dma_start(out=t_t[:], in_=t[c, :, :])
                # col 6+c: dot_c = sum(x_c * a)
                nc.vector.scalar_tensor_tensor(
                    junk[:], x_t[:], 0.0, a_t[:],
                    Al.bypass, Al.mult, accum_out=o[:, 6 + c:7 + c])
                if c < 3:
                    lnp = pool.tile([P, F], f32, tag="lnp", bufs=2)
                    nc.scalar.activation(lnp[:], x_t[:], Ln)
                    ln1p = pool.tile([P, F], f32, tag="ln1p", bufs=2)
                    nc.scalar.activation(ln1p[:], x_t[:], Ln, bias=1.0, scale=-1.0)
                    # col c: sum(t_c * ln p)
                    nc.vector.scalar_tensor_tensor(
                        junk[:], t_t[:], 0.0, lnp[:],
                        Al.bypass, Al.mult, accum_out=o[:, c:c + 1])
                    # col 3+c: sum((t_c - 1) * ln(1-p)) = -sum((1-t_c) ln(1-p))
                    nc.vector.scalar_tensor_tensor(
                        junk[:], t_t[:], 1.0, ln1p[:],
                        Al.subtract, Al.mult, accum_out=o[:, 3 + c:4 + c])
                else:
                    # col 10: n_cent partial = sum(t_3)
                    nc.vector.tensor_reduce(
                        o[:, 10:11], t_t[:], axis=mybir.AxisListType.X, op=Al.add)
            nc.sync.dma_start(out=out[:, :], in_=o[:, :])
    _split_multi_waits(nc)
    return nc


def _host_a_vector(cm):
    """Dense A with A[j] = sum_{centroid i, tap k: i+off_k=j} cm_i * w_k / cnt_i."""
    offsets, weights = _offsets_and_weights()
    A = np.zeros(N, dtype=np.float64)
    idx = np.nonzero(cm != 0.0)[0]
    for i in idx:
        ni = i + offsets
        valid = (ni >= 0) & (ni < N)
        cnt = float(valid.sum())
        A[ni[valid]] += (cm[i] / max(cnt, 1.0)) * weights[valid].astype(np.float64)
    return A.astype(np.float32)


def kernel(inputs: np.ndarray, targets: np.ndarray) -> np.ndarray:
    x_full = np.ascontiguousarray(np.asarray(inputs, dtype=np.float32).reshape(CH, N))
    t_full = np.ascontiguousarray(np.asarray(targets, dtype=np.float32).reshape(CH, N))

    A = _host_a_vector(t_full[3])

    in_maps = []
    for i in range(NCORES):
        sl = slice(i * CHUNK, (i + 1) * CHUNK)
        in_maps.append({
            "x": np.ascontiguousarray(x_full[:, sl]).reshape(CH, P, F),
            "t": np.ascontiguousarray(t_full[:, sl]).reshape(CH, P, F),
            "a": np.ascontiguousarray(A[sl]).reshape(P, F),
        })

    if "nc" not in _cache:
        _cache["nc"] = _build_nc()
    nc = _cache["nc"]

    trace = bool(int(os.environ.get("KERNEL_TRACE", "0")))
    res = bass_utils.run_bass_kernel_spmd(
        nc, in_maps, core_ids=list(range(NCORES)), trace=trace)
    kernel._last_results = res

    r = np.zeros(12, dtype=np.float64)
    for m in res.results:
        r += m["out"].astype(np.float64).sum(axis=0)

    sum_bce = r[3:6] - r[0:3]                       # sum of -(t lnp + (1-t) ln1p)
    chan_losses = sum_bce / N * np.asarray(CHAN_WEIGHTS, dtype=np.float64)
    loss = chan_losses.mean()
    n_cent = max(r[10], 1.0)
    sims = r[6:10] / n_cent
    result = loss + sims[:3].mean() * PHI + (1.0 - sims[3]) * ETA
    return np.asarray(result, dtype=np.float32)


# revision 7
# speedup vs baseline: 1.0907x; 1.0907x over previous
"""Trainium2 Bass kernel for nn_CentroidLoss (BCE + sparse-centroid selem similarity).

Full inputs in, full (scalar) output out. Sharding: flattened voxel axis N
split contiguously across 8 cores (one D-slice each).

Math: loss = mean_c BCE(x_c, t_c) + 0.5*mean(sims[:3]) + 0.5*(1-sims[3]) where
sims_c = (1/n_cent) * sum_i cm_i * (sum_k w_k valid x_c[i+off_k]) / cnt_i.
The centroid mask cm is ~0.01% dense, so the double sum is re-associated as
dot(x_c, A) with A[j] = sum_{i,k: i+off_k=j} cm_i * w_k / cnt_i, a sparse
scatter computed on host from the mask (~82*243 scalar ops). The device
streams x, t and A once (memory-roofline) and emits per-partition partial
sums; host combines the 8*128 partials into the scalar loss.
"""

import os
import numpy as np

import concourse.bass as bass
import concourse.mybir as mybir
from concourse.tile import TileContext
from concourse import bass_utils

# ---- hardcoded problem geometry ----
D, H, W = 8, 320, 320
N = D * H * W                      # 819200
NCORES = 8
CHUNK = N // NCORES                # 102400
P = 128
F = CHUNK // P                     # 800
CH = 4
EPS = 1e-7
ETA = 0.5
PHI = 0.5
CHAN_WEIGHTS = (1.0, 1.0, 1.0)

SELEM_SHAPE = (3, 9, 9)
CENTRE = (1, 4, 4)

_cache = {}


def _split_multi_waits(nc):
    """This walrus build rejects >1 sync-wait per instruction ("Too many sync
    wait commands"). Tile coalesces waits; redistribute extras onto NoOps
    inserted immediately before, on the same engine (engine blocks on each
    wait in turn — semantics preserved)."""
    n_split = 0
    for fn in nc.m.functions:
        for b in fn.blocks:
            insts = b.instructions
            i = 0
            while i < len(insts):
                inst = insts[i]
                si = getattr(inst, 'sync_info', None)
                if si is None or not si.on_wait or len(si.on_wait) <= 1:
                    i += 1
                    continue
                waits = list(si.on_wait)
                new_nops = [
                    mybir.InstNoOp(
                        name=f"{inst.name}-waitsplit-{k}",
                        engine=inst.engine,
                        sync_info=mybir.SyncInfo(on_wait=[w], on_update=[]),
                    )
                    for k, w in enumerate(waits[:-1])
                ]
                si.on_wait = [waits[-1]]
                for k, nop in enumerate(new_nops):
                    insts.insert(i + k, nop)
                i += len(new_nops) + 1
                n_split += 1
    return n_split


def _offsets_and_weights():
    idx = np.stack(np.nonzero(np.ones(SELEM_SHAPE)), axis=-1)      # (243, 3)
    disp = idx - np.asarray(CENTRE)
    strides = np.array([H * W, W, 1])
    offsets = disp @ strides                                        # (243,)
    dist = np.linalg.norm(disp.astype(np.float64), axis=1)
    weights = (dist / dist.max() - 1.0).astype(np.float32)          # (243,)
    return offsets.astype(np.int64), weights


def _build_nc():
    nc = bass.Bass()
    f32 = mybir.dt.float32
    bf16 = mybir.dt.bfloat16
    # channel-interleaved per-core layout: partition-major, then channel
    x = nc.dram_tensor("x", (P, CH, F), f32, kind="ExternalInput")
    t = nc.dram_tensor("t", (P, CH, F), bf16, kind="ExternalInput")
    a = nc.dram_tensor("a", (P, F), f32, kind="ExternalInput")
    out = nc.dram_tensor("out", (P, 12), f32, kind="ExternalOutput")
    Ln = mybir.ActivationFunctionType.Ln
    Ident = mybir.ActivationFunctionType.Identity
    Al = mybir.AluOpType

    with TileContext(nc) as tc:
        with tc.tile_pool(name="pool", bufs=1) as pool:
            o = pool.tile([P, 12], f32)
            # prewarm the Ln activation table while DMAs are in flight
            warm = pool.tile([P, 1], f32)
            nc.gpsimd.memset(warm[:], 0.5)
            nc.scalar.activation(warm[:], warm[:], Ln)
            # loads: a first (dots), x split in halves so ACT starts early
            a_t = pool.tile([P, F], f32)
            nc.sync.dma_start(out=a_t[:], in_=a[:, :])
            x_t = pool.tile([P, CH, F], f32)
            nc.sync.dma_start(out=x_t[:, 0:2, :], in_=x[:, 0:2, :])
            nc.sync.dma_start(out=x_t[:, 2:4, :], in_=x[:, 2:4, :])
            t_t = pool.tile([P, CH, F], bf16)
            nc.sync.dma_start(out=t_t[:], in_=t[:, :, :])
            junkv = pool.tile([P, F], f32)
            junkg = pool.tile([P, F], f32)
            junks = pool.tile([P, F], f32)
            lnps, ln1ps = [], []
            for c in range(3):
                lnp_c = pool.tile([P, F], f32, name=f"lnp{c}")
                nc.scalar.activation(lnp_c[:], x_t[:, c, :], Ln)
                ln1p_c = pool.tile([P, F], f32, name=f"ln1p{c}")
                nc.scalar.activation(ln1p_c[:], x_t[:, c, :], Ln,
                                     bias=1.0, scale=-1.0)
                lnps.append(lnp_c)
                ln1ps.append(ln1p_c)
            # vector: dot0 (needs only a+x01), then the 6 BCE reduces
            nc.vector.scalar_tensor_tensor(
                junkv[:], x_t[:, 0, :], 0.0, a_t[:],
                Al.bypass, Al.mult, accum_out=o[:, 6:7])
            for c in range(3):
                # col c: sum(t_c * ln p)
                nc.vector.scalar_tensor_tensor(
                    junkv[:], t_t[:, c, :], 0.0, lnps[c][:],
                    Al.bypass, Al.mult, accum_out=o[:, c:c + 1])
                # col 3+c: sum((t_c - 1) * ln(1-p))
                nc.vector.scalar_tensor_tensor(
                    junkv[:], t_t[:, c, :], 1.0, ln1ps[c][:],
                    Al.subtract, Al.mult, accum_out=o[:, 3 + c:4 + c])
            # dots for channels 1..3 (vector; Pool can't run TensorScalarPtr
            # under this walrus)
            for c in (1, 2, 3):
                nc.vector.scalar_tensor_tensor(
                    junkg[:], x_t[:, c, :], 0.0, a_t[:],
                    Al.bypass, Al.mult, accum_out=o[:, 6 + c:7 + c])
            # scalar: col 10 = n_cent partial = sum(t_3)
            nc.scalar.activation(junks[:], t_t[:, 3, :], Ident,
                                 accum_out=o[:, 10:11])
            nc.sync.dma_start(out=out[:, :], in_=o[:, :])
    _split_multi_waits(nc)
    return nc


def _host_a_vector(cm):
    """Dense A with A[j] = sum_{centroid i, tap k: i+off_k=j} cm_i * w_k / cnt_i."""
    offsets, weights = _offsets_and_weights()
    A = np.zeros(N, dtype=np.float64)
    idx = np.nonzero(cm != 0.0)[0]
    for i in idx:
        ni = i + offsets
        valid = (ni >= 0) & (ni < N)
        cnt = float(valid.sum())
        A[ni[valid]] += (cm[i] / max(cnt, 1.0)) * weights[valid].astype(np.float64)
    return A.astype(np.float32)


def kernel(inputs: np.ndarray, targets: np.ndarray) -> np.ndarray:
    x_full = np.ascontiguousarray(np.asarray(inputs, dtype=np.float32).reshape(CH, N))
    t_full = np.ascontiguousarray(np.asarray(targets, dtype=np.float32).reshape(CH, N))

    A = _host_a_vector(t_full[3])

    import ml_dtypes
    in_maps = []
    for i in range(NCORES):
        sl = slice(i * CHUNK, (i + 1) * CHUNK)
        x_sh = x_full[:, sl].reshape(CH, P, F).transpose(1, 0, 2)
        t_sh = t_full[:, sl].reshape(CH, P, F).transpose(1, 0, 2)
        in_maps.append({
            "x": np.ascontiguousarray(x_sh),
            "t": np.ascontiguousarray(t_sh).astype(ml_dtypes.bfloat16),
            "a": np.ascontiguousarray(A[sl]).reshape(P, F),
        })

    if "nc" not in _cache:
        _cache["nc"] = _build_nc()
    nc = _cache["nc"]

    trace = bool(int(os.environ.get("KERNEL_TRACE", "0")))
    res = bass_utils.run_bass_kernel_spmd(
        nc, in_maps, core_ids=list(range(NCORES)), trace=trace)
    kernel._last_results = res

    r = np.zeros(12, dtype=np.float64)
    for m in res.results:
        r += m["out"].astype(np.float64).sum(axis=0)

    sum_bce = r[3:6] - r[0:3]                       # sum of -(t lnp + (1-t) ln1p)
    chan_losses = sum_bce / N * np.asarray(CHAN_WEIGHTS, dtype=np.float64)
    loss = chan_losses.mean()
    n_cent = max(r[10], 1.0)
    sims = r[6:10] / n_cent
    result = loss + sims[:3].mean() * PHI + (1.0 - sims[3]) * ETA
    return np.asarray(result, dtype=np.float32)
